# revision 52
# baseline (speedup 1.0000x reference)
"""Distributed LlamaAttention (B=2, S=2048, H=2048, 16 heads) on one TRN2 chip.

Sharding: tensor-parallel over heads - core c owns heads (2c, 2c+1).
  * q/k/v projections -> qT/kT/vT [d, tok] (weight-stationary, N=512)
  * v to natural [tok, d] layout via XBAR DMA transpose (no PE cost)
  * attention: scores computed TRANSPOSED S_T[k,q] k-tile-outer so the
    exp'd probabilities feed the AV matmul directly (v stationary); the
    softmax denominator comes from a vector presum of the exp'd strips
    plus one all-ones matmul that column-sums AND broadcasts in one shot;
    1/denom via DVE newton-raphson (exact reciprocal is 4us/tile and
    scalar ln/exp forces 1.3us activation-table reloads)
  * o-projection: row-parallel -> per-core partials; host sums the 8

All matmuls bf16 (f32 for the tiny denominator reduction) with f32 PSUM.
Self-contained: hardcodes all shapes; no sibling imports.
"""

import math

import numpy as np
import ml_dtypes

B, S, HIDDEN, NH, HD = 2, 2048, 2048, 16, 128
N_CORES = 8
HPC = NH // N_CORES          # heads per core = 2
M = HPC * HD                 # per-core projection width = 256
T = B * S                    # 4096 tokens
P = 128                      # partitions
TCH = 512                    # token / free-dim chunk
NTCH = T // TCH              # 8
QT = S // P                  # 16 token tiles per batch
KI = HIDDEN // P             # 16 contraction tiles for projections
NQ = 4                       # query quarters per (b,h); each 4 tiles = 512 tok
BF16 = ml_dtypes.bfloat16

_nc_cache = {}


def _build_nc():
    import concourse.bacc as bacc
    import concourse.mybir as mybir
    from concourse import tile
    from contextlib import ExitStack

    bf = mybir.dt.bfloat16
    f32 = mybir.dt.float32
    AF = mybir.ActivationFunctionType

    nc = bacc.Bacc("TRN2", target_bir_lowering=False, debug=False)

    # inputs host-prearranged to partition-major so DMA descriptors are
    # 4-16KB contiguous per partition (strided loads run ~3x slower)
    hsT = nc.dram_tensor("hsT", [P, NTCH, KI, TCH], bf, kind="ExternalInput").ap()
    wqT = nc.dram_tensor("wqT", [P, KI, M], bf, kind="ExternalInput").ap()
    wkT = nc.dram_tensor("wkT", [P, KI, M], bf, kind="ExternalInput").ap()
    wvT = nc.dram_tensor("wvT", [P, KI, M], bf, kind="ExternalInput").ap()
    woT = nc.dram_tensor("woT", [P, HPC, HIDDEN], bf, kind="ExternalInput").ap()
    msk = nc.dram_tensor("mask", [P, P], f32, kind="ExternalInput").ap()
    out = nc.dram_tensor("out", [T, HIDDEN], bf, kind="ExternalOutput").ap()

    out_r = out.rearrange("(n p) o -> p n o", p=P)      # [128, 32, 2048]

    inv_sqrt_d = 1.0 / math.sqrt(HD)

    with tile.TileContext(nc) as tc, ExitStack() as ctx:
        const = ctx.enter_context(tc.tile_pool(name="const", bufs=1))
        qkv = ctx.enter_context(tc.tile_pool(name="qkv", bufs=1))
        hsp = ctx.enter_context(tc.tile_pool(name="hsp", bufs=3))
        ppl = ctx.enter_context(tc.tile_pool(name="ppl", bufs=6))
        pac = ctx.enter_context(tc.tile_pool(name="pac", bufs=2))
        rcp = ctx.enter_context(tc.tile_pool(name="rcp", bufs=2))
        orp = ctx.enter_context(tc.tile_pool(name="orp", bufs=3))
        pp = ctx.enter_context(tc.tile_pool(name="pp", bufs=2, space="PSUM"))
        stp = ctx.enter_context(tc.tile_pool(name="stp", bufs=3, space="PSUM"))
        avp = ctx.enter_context(tc.tile_pool(name="avp", bufs=2, space="PSUM"))
        bcp = ctx.enter_context(tc.tile_pool(name="bcp", bufs=1, space="PSUM"))

        # --- constants / weights resident in SBUF ---
        # DMA order tuned for startup: wq first (chunk 0's q matmuls need
        # only wq + first hs half); wk/wv after chunk 0's hs DMA; wo (only
        # needed by phase 3) deferred until after chunk 0 is emitted.
        wq_sb = const.tile([P, KI, M], bf)
        wk_sb = const.tile([P, KI, M], bf)
        wv_sb = const.tile([P, KI, M], bf)
        wo_sb = const.tile([P, HPC, HIDDEN], bf)
        msk_sb = const.tile([P, P], f32)
        ones_sb = const.tile([P, P], bf)
        for i4 in range(0, KI, 4):
            nc.sync.dma_start(wq_sb[:, i4:i4 + 4], wqT[:, i4:i4 + 4])
        nc.vector.memset(ones_sb[:], 1.0)

        # --- persistent activations ---
        qT_b = [qkv.tile([P, HPC, S], bf, tag=f"qT{b}", name=f"qT{b}") for b in range(B)]
        kT_b = [qkv.tile([P, HPC, S], bf, tag=f"kT{b}", name=f"kT{b}") for b in range(B)]
        vT_b = [qkv.tile([P, HPC, S], bf, tag=f"vT{b}", name=f"vT{b}") for b in range(B)]
        cxT_b = [qkv.tile([P, HPC, S], bf, tag=f"cxT{b}", name=f"cxT{b}") for b in range(B)]
        # v natural layout: [tok%128, head, tile, d]
        v_b = [qkv.tile([P, HPC, QT, P], bf, tag=f"v{b}", name=f"v{b}") for b in range(B)]

        # ---- Phase 1 emitter: one 512-token chunk; yields per psum-group ----
        def p1_chunk(j):
            hs_t = hsp.tile([P, KI, TCH], bf, tag="hs", name="hs_t")
            # chunk 0: finely split on the second hwdge queue so the first
            # matmul starts after a quarter of wq + a quarter of hs
            if j == 0:
                for i4 in range(0, KI, 4):
                    nc.scalar.dma_start(hs_t[:, i4:i4 + 4], hsT[:, j, i4:i4 + 4])
            else:
                nc.sync.dma_start(hs_t[:, :KI // 2], hsT[:, j, :KI // 2])
                nc.sync.dma_start(hs_t[:, KI // 2:], hsT[:, j, KI // 2:])
            b = j // 4
            joff = (j % 4) * TCH
            # q projection (scale 1/sqrt(d) folded into the copy)
            for h in range(HPC):
                ps = pp.tile([P, TCH], f32, tag="pp", name="ps_q")
                for i in range(KI):
                    nc.tensor.matmul(
                        ps[:], wq_sb[:, i, h * P:(h + 1) * P], hs_t[:, i, :],
                        start=(i == 0), stop=(i == KI - 1),
                    )
                if h == 0:
                    nc.scalar.mul(qT_b[b][:, h, joff:joff + TCH], ps[:], inv_sqrt_d)
                else:
                    nc.vector.tensor_scalar_mul(
                        qT_b[b][:, h, joff:joff + TCH], ps[:], inv_sqrt_d)
                yield
            # k projection
            for h in range(HPC):
                ps = pp.tile([P, TCH], f32, tag="pp", name="ps_k")
                for i in range(KI):
                    nc.tensor.matmul(
                        ps[:], wk_sb[:, i, h * P:(h + 1) * P], hs_t[:, i, :],
                        start=(i == 0), stop=(i == KI - 1),
                    )
                if h == 0:
                    nc.scalar.copy(kT_b[b][:, h, joff:joff + TCH], ps[:])
                else:
                    nc.vector.tensor_copy(kT_b[b][:, h, joff:joff + TCH], ps[:])
                yield
            # v projection in [d, tok]; then XBAR transpose to [tok, d]
            for h in range(HPC):
                ps = pp.tile([P, TCH], f32, tag="pp", name="ps_v")
                for i in range(KI):
                    nc.tensor.matmul(
                        ps[:], wv_sb[:, i, h * P:(h + 1) * P], hs_t[:, i, :],
                        start=(i == 0), stop=(i == KI - 1),
                    )
                if h == 0:
                    nc.scalar.copy(vT_b[b][:, h, joff:joff + TCH], ps[:])
                else:
                    nc.vector.tensor_copy(vT_b[b][:, h, joff:joff + TCH], ps[:])
                nc.sync.dma_start_transpose(
                    v_b[b][:, h, (j % 4) * 4:(j % 4) * 4 + 4, :],
                    vT_b[b][:, h, joff:joff + TCH],
                )
                yield

        # ---- Phase 2 emitter: attention for (b, h); yields per k-tile ----
        def attn_quarter(b, h, Q):
            qT, kT, cxT, v = qT_b[b], kT_b[b], cxT_b[b], v_b[b]
            if True:
                q0 = Q * 4 * P                       # quarter col offset (tokens)
                p_acc = pac.tile([P, TCH], f32, tag="pacc", name="p_acc")
                p_accb = pac.tile([P, TCH], bf, tag="paccb", name="p_accb")
                av = avp.tile([P, TCH], f32, tag="av", name="av")
                p0 = None                            # kb=0 strip, fused into kb=1 add
                nkb = 4 * Q + 4
                pend = []                            # skewed AV emission
                for kb in range(nkb):
                    c0 = max(kb - 4 * Q, 0) * P      # first live col in quarter
                    st = stp.tile([P, TCH], f32, tag="st", name="st")
                    nc.tensor.matmul(
                        st[:, c0:], kT[:, h, kb * P:(kb + 1) * P],
                        qT[:, h, q0 + c0:q0 + 4 * P],
                        start=True, stop=True,
                    )
                    if kb >= 4 * Q:                  # causal diag tile mask
                        nc.vector.tensor_add(st[:, c0:c0 + P], st[:, c0:c0 + P], msk_sb[:])
                    p = ppl.tile([P, TCH], bf, tag="p", name="p_t")
                    nc.scalar.activation(p[:, c0:], st[:, c0:], AF.Exp)
                    # presum; kb=0's copy is fused into kb=1's add
                    if kb == 0:
                        p0 = p
                    elif kb == 1:
                        nc.vector.tensor_add(p_acc[:, c0:], p0[:, c0:], p[:, c0:])
                        if c0:
                            nc.vector.tensor_copy(p_acc[:, :c0], p0[:, :c0])
                    else:
                        nc.vector.tensor_add(p_acc[:, c0:], p_acc[:, c0:], p[:, c0:])
                    if len(pend) >= 1:
                        yield from pend.pop(0)
                    def av_mm(kb=kb, c0=c0, p=p):
                        nc.tensor.matmul(
                            av[:, c0:], v[:, h, kb, :], p[:, c0:],
                            start=(kb == 0), stop=(kb == nkb - 1),
                        )
                        yield
                    pend.append(av_mm())
                for g in pend:
                    yield from g
                # denominator: column-sum + partition-broadcast in one matmul;
                # 1/d via single-op newton-raphson on DVE (~18 bits, plenty)
                # bf16 cast on scalar so the colsum matmul streams at bf16
                # rate (f32 matmuls are ~3x slower per column)
                nc.scalar.copy(p_accb[:], p_acc[:])
                bc = bcp.tile([P, TCH], f32, tag="bc", name="bc")
                nc.tensor.matmul(bc[:], ones_sb[:], p_accb[:], start=True, stop=True)
                rc = rcp.tile([P, TCH], f32, tag="rc", name="rc")
                nc.vector.reciprocal_approx_fast(rc[:], bc[:])
                nc.vector.tensor_mul(cxT[:, h, q0:q0 + 4 * P], av[:], rc[:])
                yield

        def attn_pair(b, h):
            for Q in range(NQ):
                yield from attn_quarter(b, h, Q)

        # ---- Phase 3 emitter: o-projection rows; yields per token tile ----
        def p3_rows(b):
            for tloc in range(QT):
                orow = orp.tile([P, HIDDEN], bf, tag="orow", name="orow")
                for oc in range(HIDDEN // TCH):
                    ps = pp.tile([P, TCH], f32, tag="pp", name="ps_o")
                    for h in range(HPC):
                        nc.tensor.matmul(
                            ps[:], cxT_b[b][:, h, tloc * P:(tloc + 1) * P],
                            wo_sb[:, h, oc * TCH:(oc + 1) * TCH],
                            start=(h == 0), stop=(h == HPC - 1),
                        )
                    if oc % 2 == 0:
                        nc.scalar.copy(orow[:, oc * TCH:(oc + 1) * TCH], ps[:])
                    else:
                        nc.vector.tensor_copy(orow[:, oc * TCH:(oc + 1) * TCH], ps[:])
                nc.sync.dma_start(out_r[:, b * QT + tloc, :], orow[:])
                yield

        def run(gen):
            for _ in gen:
                pass

        def interleave(main, filler, ratio):
            """Drive main; after every `ratio` main steps, one filler step."""
            n = 0
            for _ in main:
                n += 1
                if filler is not None and n % ratio == 0:
                    try:
                        next(filler)
                    except StopIteration:
                        filler = None
            while filler is not None:
                try:
                    next(filler)
                except StopIteration:
                    filler = None

        def interleave_gated(main, filler, gates, spread=2):
            """Drive main; `gates[n]` releases that many filler steps once
            main has taken n steps (at most one per `spread` main steps)."""
            n, budget = 0, 0
            for _ in main:
                n += 1
                budget += gates.get(n, 0)
                if filler is not None and budget > 0 and n % spread == 0:
                    budget -= 1
                    try:
                        next(filler)
                    except StopIteration:
                        filler = None
            while filler is not None:
                try:
                    next(filler)
                except StopIteration:
                    filler = None

        def chain(*gens):
            for g in gens:
                yield from g

        # schedule: P1(b0); A(b0)+P1(b1); A(b1 both heads, quarter-interleaved,
        # with h1's last quarter BEFORE h0's so phase 3 tiles unlock early)+P3
        # chunk 0 starts on wq + its first hs half; wk/wv land during its
        # q-groups, wo during chunk 1
        g0 = p1_chunk(0)
        next(g0)
        next(g0)
        nc.sync.dma_start(wk_sb[:], wkT)
        nc.sync.dma_start(wv_sb[:], wvT)
        nc.sync.dma_start(msk_sb[:], msk)
        run(g0)
        nc.sync.dma_start(wo_sb[:], woT)
        for j in range(1, 4):
            run(p1_chunk(j))
        interleave(chain(attn_pair(0, 0), attn_pair(0, 1)),
                   chain(*[p1_chunk(j) for j in range(4, 8)]), ratio=3)
        interleave(attn_pair(1, 0), p3_rows(0), ratio=3)
        # p3(b1) token tiles 4Q..4Q+3 unlock after quarter Q of the last pair
        interleave_gated(attn_pair(1, 1), p3_rows(1),
                         gates={5: 4, 14: 4, 27: 4, 44: 4}, spread=1)

    nc.compile()
    return nc


def get_nc():
    if "nc" not in _nc_cache:
        _nc_cache["nc"] = _build_nc()
    return _nc_cache["nc"]


def _warr(wT):
    """[HIDDEN, M] transposed weight -> partition-major [P, KI, M]."""
    return np.ascontiguousarray(
        wT.reshape(KI, P, M).transpose(1, 0, 2)).astype(BF16)


def make_in_maps(hidden_states, wq, wk, wv, wo):
    hs = np.asarray(hidden_states, dtype=np.float32).reshape(T, HIDDEN)
    # [hid, tok] -> partition-major chunks [P, NTCH, KI, TCH]
    hsT = np.ascontiguousarray(
        hs.T.reshape(KI, P, NTCH, TCH).transpose(1, 2, 0, 3)).astype(BF16)
    # S_T[k, q] layout: mask out k > q (strictly lower triangle)
    mask = np.tril(np.full((P, P), -1e9, dtype=np.float32), -1)
    wq = np.asarray(wq, dtype=np.float32)
    wk = np.asarray(wk, dtype=np.float32)
    wv = np.asarray(wv, dtype=np.float32)
    wo = np.asarray(wo, dtype=np.float32)
    in_maps = []
    for c in range(N_CORES):
        sl = slice(c * M, (c + 1) * M)
        woc = np.ascontiguousarray(
            wo[:, sl].T.reshape(HPC, P, HIDDEN).transpose(1, 0, 2)).astype(BF16)
        in_maps.append({
            "hsT": hsT,
            "wqT": _warr(wq[sl, :].T),
            "wkT": _warr(wk[sl, :].T),
            "wvT": _warr(wv[sl, :].T),
            "woT": woc,
            "mask": mask,
        })
    return in_maps


def kernel(hidden_states, wq, wk, wv, wo):
    from concourse.bass_utils import run_bass_kernel_spmd

    nc = get_nc()
    in_maps = make_in_maps(hidden_states, wq, wk, wv, wo)
    res = run_bass_kernel_spmd(nc, in_maps, core_ids=list(range(N_CORES)))
    acc = np.zeros((T, HIDDEN), dtype=np.float32)
    for r in res.results:
        acc += np.asarray(r["out"]).astype(np.float32)
    return acc.reshape(B, S, HIDDEN)


# revision 54
# speedup vs baseline: 1.1969x; 1.1969x over previous
"""Distributed LlamaAttention (B=2, S=2048, H=2048, 16 heads) on one TRN2 chip.

Sharding: tensor-parallel over heads - core c owns heads (2c, 2c+1).
  * q/k/v projections -> qT/kT/vT [d, tok] (weight-stationary, N=512)
  * v to natural [tok, d] layout via XBAR DMA transpose (no PE cost)
  * attention: scores computed TRANSPOSED S_T[k,q] k-tile-outer so the
    exp'd probabilities feed the AV matmul directly (v stationary); the
    softmax denominator comes from a vector presum of the exp'd strips
    plus one all-ones matmul that column-sums AND broadcasts in one shot;
    1/denom via DVE newton-raphson (exact reciprocal is 4us/tile and
    scalar ln/exp forces 1.3us activation-table reloads)
  * o-projection: row-parallel -> per-core partials; host sums the 8

All matmuls bf16 (f32 for the tiny denominator reduction) with f32 PSUM.
Self-contained: hardcodes all shapes; no sibling imports.
"""

import math

import numpy as np
import ml_dtypes

B, S, HIDDEN, NH, HD = 2, 2048, 2048, 16, 128
N_CORES = 8
HPC = NH // N_CORES          # heads per core = 2
M = HPC * HD                 # per-core projection width = 256
T = B * S                    # 4096 tokens
P = 128                      # partitions
TCH = 512                    # token / free-dim chunk
NTCH = T // TCH              # 8
QT = S // P                  # 16 token tiles per batch
KI = HIDDEN // P             # 16 contraction tiles for projections
NQ = 4                       # query quarters per (b,h); each 4 tiles = 512 tok
BF16 = ml_dtypes.bfloat16

_nc_cache = {}


def _build_nc():
    import concourse.bacc as bacc
    import concourse.mybir as mybir
    from concourse import tile
    from contextlib import ExitStack

    bf = mybir.dt.bfloat16
    f32 = mybir.dt.float32
    AF = mybir.ActivationFunctionType

    nc = bacc.Bacc("TRN2", target_bir_lowering=False, debug=False)

    # inputs host-prearranged to partition-major so DMA descriptors are
    # 4-16KB contiguous per partition (strided loads run ~3x slower)
    hsT = nc.dram_tensor("hsT", [P, NTCH, KI, TCH], bf, kind="ExternalInput").ap()
    wqT = nc.dram_tensor("wqT", [P, KI, M], bf, kind="ExternalInput").ap()
    wkT = nc.dram_tensor("wkT", [P, KI, M], bf, kind="ExternalInput").ap()
    wvT = nc.dram_tensor("wvT", [P, KI, M], bf, kind="ExternalInput").ap()
    woT = nc.dram_tensor("woT", [P, HPC, HIDDEN], bf, kind="ExternalInput").ap()
    msk = nc.dram_tensor("mask", [P, P], f32, kind="ExternalInput").ap()
    out = nc.dram_tensor("out", [T, HIDDEN], bf, kind="ExternalOutput").ap()

    out_r = out.rearrange("(n p) o -> p n o", p=P)      # [128, 32, 2048]

    inv_sqrt_d = 1.0 / math.sqrt(HD)

    with tile.TileContext(nc) as tc, ExitStack() as ctx:
        const = ctx.enter_context(tc.tile_pool(name="const", bufs=1))
        qkv = ctx.enter_context(tc.tile_pool(name="qkv", bufs=1))
        hsp = ctx.enter_context(tc.tile_pool(name="hsp", bufs=3))
        ppl = ctx.enter_context(tc.tile_pool(name="ppl", bufs=6))
        pac = ctx.enter_context(tc.tile_pool(name="pac", bufs=2))
        rcp = ctx.enter_context(tc.tile_pool(name="rcp", bufs=2))
        orp = ctx.enter_context(tc.tile_pool(name="orp", bufs=3))
        pp = ctx.enter_context(tc.tile_pool(name="pp", bufs=2, space="PSUM"))
        stp = ctx.enter_context(tc.tile_pool(name="stp", bufs=3, space="PSUM"))
        avp = ctx.enter_context(tc.tile_pool(name="avp", bufs=2, space="PSUM"))
        bcp = ctx.enter_context(tc.tile_pool(name="bcp", bufs=1, space="PSUM"))

        # --- constants / weights resident in SBUF ---
        # DMA order tuned for startup: wq first (chunk 0's q matmuls need
        # only wq + first hs half); wk/wv after chunk 0's hs DMA; wo (only
        # needed by phase 3) deferred until after chunk 0 is emitted.
        wq_sb = const.tile([P, KI, M], bf)
        wk_sb = const.tile([P, KI, M], bf)
        wv_sb = const.tile([P, KI, M], bf)
        wo_sb = const.tile([P, HPC, HIDDEN], bf)
        msk_sb = const.tile([P, P], f32)
        ones_sb = const.tile([P, P], bf)
        for i4 in range(0, KI, 4):
            nc.sync.dma_start(wq_sb[:, i4:i4 + 4], wqT[:, i4:i4 + 4])
        nc.vector.memset(ones_sb[:], 1.0)

        # --- persistent activations ---
        qT_b = [qkv.tile([P, HPC, S], bf, tag=f"qT{b}", name=f"qT{b}") for b in range(B)]
        kT_b = [qkv.tile([P, HPC, S], bf, tag=f"kT{b}", name=f"kT{b}") for b in range(B)]
        vT_b = [qkv.tile([P, HPC, S], bf, tag=f"vT{b}", name=f"vT{b}") for b in range(B)]
        cxT_b = [qkv.tile([P, HPC, S], bf, tag=f"cxT{b}", name=f"cxT{b}") for b in range(B)]
        # v natural layout: [tok%128, head, tile, d]
        v_b = [qkv.tile([P, HPC, QT, P], bf, tag=f"v{b}", name=f"v{b}") for b in range(B)]

        # ---- Phase 1 emitter: one 512-token chunk; yields per psum-group ----
        def p1_chunk(j):
            hs_t = hsp.tile([P, KI, TCH], bf, tag="hs", name="hs_t")
            # chunk 0: finely split on the second hwdge queue so the first
            # matmul starts after a quarter of wq + a quarter of hs
            if j == 0:
                for i4 in range(0, KI, 4):
                    nc.scalar.dma_start(hs_t[:, i4:i4 + 4], hsT[:, j, i4:i4 + 4])
            else:
                nc.sync.dma_start(hs_t[:, :KI // 2], hsT[:, j, :KI // 2])
                nc.sync.dma_start(hs_t[:, KI // 2:], hsT[:, j, KI // 2:])
            b = j // 4
            joff = (j % 4) * TCH
            # q projection (scale 1/sqrt(d) folded into the copy)
            for h in range(HPC):
                ps = pp.tile([P, TCH], f32, tag="pp", name="ps_q")
                for i in range(KI):
                    nc.tensor.matmul(
                        ps[:], wq_sb[:, i, h * P:(h + 1) * P], hs_t[:, i, :],
                        start=(i == 0), stop=(i == KI - 1),
                    )
                if h == 0:
                    nc.scalar.mul(qT_b[b][:, h, joff:joff + TCH], ps[:], inv_sqrt_d)
                else:
                    nc.vector.tensor_scalar_mul(
                        qT_b[b][:, h, joff:joff + TCH], ps[:], inv_sqrt_d)
                yield
            # k projection
            for h in range(HPC):
                ps = pp.tile([P, TCH], f32, tag="pp", name="ps_k")
                for i in range(KI):
                    nc.tensor.matmul(
                        ps[:], wk_sb[:, i, h * P:(h + 1) * P], hs_t[:, i, :],
                        start=(i == 0), stop=(i == KI - 1),
                    )
                if h == 0:
                    nc.scalar.copy(kT_b[b][:, h, joff:joff + TCH], ps[:])
                else:
                    nc.vector.tensor_copy(kT_b[b][:, h, joff:joff + TCH], ps[:])
                yield
            # v projection in [d, tok]; then XBAR transpose to [tok, d]
            for h in range(HPC):
                ps = pp.tile([P, TCH], f32, tag="pp", name="ps_v")
                for i in range(KI):
                    nc.tensor.matmul(
                        ps[:], wv_sb[:, i, h * P:(h + 1) * P], hs_t[:, i, :],
                        start=(i == 0), stop=(i == KI - 1),
                    )
                if h == 0:
                    nc.scalar.copy(vT_b[b][:, h, joff:joff + TCH], ps[:])
                else:
                    nc.vector.tensor_copy(vT_b[b][:, h, joff:joff + TCH], ps[:])
                nc.sync.dma_start_transpose(
                    v_b[b][:, h, (j % 4) * 4:(j % 4) * 4 + 4, :],
                    vT_b[b][:, h, joff:joff + TCH],
                )
                yield

        # ---- Phase 2 emitter: attention for (b, h); yields per k-tile ----
        def attn_quarter(b, h, Q):
            qT, kT, cxT, v = qT_b[b], kT_b[b], cxT_b[b], v_b[b]
            if True:
                q0 = Q * 4 * P                       # quarter col offset (tokens)
                p_acc = pac.tile([P, TCH], f32, tag="pacc", name="p_acc")
                p_accb = pac.tile([P, TCH], bf, tag="paccb", name="p_accb")
                av = avp.tile([P, TCH], f32, tag="av", name="av")
                p_prev = None                        # deferred strip for pairing
                t_first = None                       # first pair-sum, fused below
                acc_live = False                     # p_acc initialized yet?
                nkb = 4 * Q + 4
                pend = []                            # skewed AV emission
                for kb in range(nkb):
                    c0 = max(kb - 4 * Q, 0) * P      # first live col in quarter
                    st = stp.tile([P, TCH], f32, tag="st", name="st")
                    nc.tensor.matmul(
                        st[:, c0:], kT[:, h, kb * P:(kb + 1) * P],
                        qT[:, h, q0 + c0:q0 + 4 * P],
                        start=True, stop=True,
                    )
                    if kb >= 4 * Q:                  # causal diag tile mask
                        nc.vector.tensor_add(st[:, c0:c0 + P], st[:, c0:c0 + P], msk_sb[:])
                    p = ppl.tile([P, TCH], bf, tag="p", name="p_t")
                    nc.scalar.activation(p[:, c0:], st[:, c0:], AF.Exp)
                    # presum: full-width strips are paired with bf16 adds
                    # (2x DVE rate); pair-sums and diag strips accumulate f32
                    if kb < 4 * Q:
                        if p_prev is None:
                            p_prev = p
                        else:
                            ts = ppl.tile([P, TCH], bf, tag="ts", name="ts", bufs=2)
                            nc.vector.tensor_add(ts[:], p_prev[:], p[:])
                            p_prev = None
                            if t_first is None:
                                t_first = ts
                            elif not acc_live:
                                nc.vector.tensor_add(p_acc[:], t_first[:], ts[:])
                                acc_live = True
                            else:
                                nc.vector.tensor_add(p_acc[:], p_acc[:], ts[:])
                    elif t_first is not None and not acc_live:
                        # Q1: single pair-sum; fold it in with this diag strip
                        nc.vector.tensor_add(p_acc[:], t_first[:], p[:])
                        acc_live = True
                    elif not acc_live:
                        # Q0: no pairs; first two diag strips fuse
                        if p_prev is None:
                            p_prev = p
                        else:
                            nc.vector.tensor_add(p_acc[:, c0:], p_prev[:, c0:], p[:, c0:])
                            if c0:
                                nc.vector.tensor_copy(p_acc[:, :c0], p_prev[:, :c0])
                            p_prev = None
                            acc_live = True
                    else:
                        nc.vector.tensor_add(p_acc[:, c0:], p_acc[:, c0:], p[:, c0:])
                    if len(pend) >= 1:
                        yield from pend.pop(0)
                    def av_mm(kb=kb, c0=c0, p=p):
                        nc.tensor.matmul(
                            av[:, c0:], v[:, h, kb, :], p[:, c0:],
                            start=(kb == 0), stop=(kb == nkb - 1),
                        )
                        yield
                    pend.append(av_mm())
                for g in pend:
                    yield from g
                # denominator: column-sum + partition-broadcast in one matmul;
                # 1/d via single-op newton-raphson on DVE (~18 bits, plenty)
                # bf16 cast on scalar so the colsum matmul streams at bf16
                # rate (f32 matmuls are ~3x slower per column)
                nc.scalar.copy(p_accb[:], p_acc[:])
                bc = bcp.tile([P, TCH], f32, tag="bc", name="bc")
                nc.tensor.matmul(bc[:], ones_sb[:], p_accb[:], start=True, stop=True)
                rc = rcp.tile([P, TCH], f32, tag="rc", name="rc")
                nc.vector.reciprocal_approx_fast(rc[:], bc[:])
                nc.vector.tensor_mul(cxT[:, h, q0:q0 + 4 * P], av[:], rc[:])
                yield

        def attn_pair(b, h):
            for Q in range(NQ):
                yield from attn_quarter(b, h, Q)

        # ---- Phase 3 emitter: o-projection rows; yields per token tile ----
        def p3_rows(b):
            for tloc in range(QT):
                orow = orp.tile([P, HIDDEN], bf, tag="orow", name="orow")
                for oc in range(HIDDEN // TCH):
                    ps = pp.tile([P, TCH], f32, tag="pp", name="ps_o")
                    for h in range(HPC):
                        nc.tensor.matmul(
                            ps[:], cxT_b[b][:, h, tloc * P:(tloc + 1) * P],
                            wo_sb[:, h, oc * TCH:(oc + 1) * TCH],
                            start=(h == 0), stop=(h == HPC - 1),
                        )
                    if oc % 2 == 0:
                        nc.scalar.copy(orow[:, oc * TCH:(oc + 1) * TCH], ps[:])
                    else:
                        nc.vector.tensor_copy(orow[:, oc * TCH:(oc + 1) * TCH], ps[:])
                nc.sync.dma_start(out_r[:, b * QT + tloc, :], orow[:])
                yield

        def run(gen):
            for _ in gen:
                pass

        def interleave(main, filler, ratio):
            """Drive main; after every `ratio` main steps, one filler step."""
            n = 0
            for _ in main:
                n += 1
                if filler is not None and n % ratio == 0:
                    try:
                        next(filler)
                    except StopIteration:
                        filler = None
            while filler is not None:
                try:
                    next(filler)
                except StopIteration:
                    filler = None

        def interleave_gated(main, filler, gates, spread=2):
            """Drive main; `gates[n]` releases that many filler steps once
            main has taken n steps (at most one per `spread` main steps)."""
            n, budget = 0, 0
            for _ in main:
                n += 1
                budget += gates.get(n, 0)
                if filler is not None and budget > 0 and n % spread == 0:
                    budget -= 1
                    try:
                        next(filler)
                    except StopIteration:
                        filler = None
            while filler is not None:
                try:
                    next(filler)
                except StopIteration:
                    filler = None

        def chain(*gens):
            for g in gens:
                yield from g

        # schedule: P1(b0); A(b0)+P1(b1); A(b1 both heads, quarter-interleaved,
        # with h1's last quarter BEFORE h0's so phase 3 tiles unlock early)+P3
        # chunk 0 starts on wq + its first hs half; wk/wv land during its
        # q-groups, wo during chunk 1
        g0 = p1_chunk(0)
        next(g0)
        next(g0)
        nc.sync.dma_start(wk_sb[:], wkT)
        nc.sync.dma_start(wv_sb[:], wvT)
        nc.sync.dma_start(msk_sb[:], msk)
        run(g0)
        nc.sync.dma_start(wo_sb[:], woT)
        for j in range(1, 4):
            run(p1_chunk(j))
        interleave(chain(attn_pair(0, 0), attn_pair(0, 1)),
                   chain(*[p1_chunk(j) for j in range(4, 8)]), ratio=3)
        interleave(attn_pair(1, 0), p3_rows(0), ratio=3)
        # p3(b1) token tiles 4Q..4Q+3 unlock after quarter Q of the last pair
        interleave_gated(attn_pair(1, 1), p3_rows(1),
                         gates={5: 4, 14: 4, 27: 4, 44: 4}, spread=1)

    nc.compile()
    return nc


def get_nc():
    if "nc" not in _nc_cache:
        _nc_cache["nc"] = _build_nc()
    return _nc_cache["nc"]


def _warr(wT):
    """[HIDDEN, M] transposed weight -> partition-major [P, KI, M]."""
    return np.ascontiguousarray(
        wT.reshape(KI, P, M).transpose(1, 0, 2)).astype(BF16)


def make_in_maps(hidden_states, wq, wk, wv, wo):
    hs = np.asarray(hidden_states, dtype=np.float32).reshape(T, HIDDEN)
    # [hid, tok] -> partition-major chunks [P, NTCH, KI, TCH]
    hsT = np.ascontiguousarray(
        hs.T.reshape(KI, P, NTCH, TCH).transpose(1, 2, 0, 3)).astype(BF16)
    # S_T[k, q] layout: mask out k > q (strictly lower triangle)
    mask = np.tril(np.full((P, P), -1e9, dtype=np.float32), -1)
    wq = np.asarray(wq, dtype=np.float32)
    wk = np.asarray(wk, dtype=np.float32)
    wv = np.asarray(wv, dtype=np.float32)
    wo = np.asarray(wo, dtype=np.float32)
    in_maps = []
    for c in range(N_CORES):
        sl = slice(c * M, (c + 1) * M)
        woc = np.ascontiguousarray(
            wo[:, sl].T.reshape(HPC, P, HIDDEN).transpose(1, 0, 2)).astype(BF16)
        in_maps.append({
            "hsT": hsT,
            "wqT": _warr(wq[sl, :].T),
            "wkT": _warr(wk[sl, :].T),
            "wvT": _warr(wv[sl, :].T),
            "woT": woc,
            "mask": mask,
        })
    return in_maps


def kernel(hidden_states, wq, wk, wv, wo):
    from concourse.bass_utils import run_bass_kernel_spmd

    nc = get_nc()
    in_maps = make_in_maps(hidden_states, wq, wk, wv, wo)
    res = run_bass_kernel_spmd(nc, in_maps, core_ids=list(range(N_CORES)))
    acc = np.zeros((T, HIDDEN), dtype=np.float32)
    for r in res.results:
        acc += np.asarray(r["out"]).astype(np.float32)
    return acc.reshape(B, S, HIDDEN)


# revision 58
# speedup vs baseline: 1.1972x; 1.0003x over previous
"""Distributed LlamaAttention (B=2, S=2048, H=2048, 16 heads) on one TRN2 chip.

Sharding: tensor-parallel over heads - core c owns heads (2c, 2c+1).
  * q/k/v projections -> qT/kT/vT [d, tok] (weight-stationary, N=512)
  * v to natural [tok, d] layout via XBAR DMA transpose (no PE cost)
  * attention: scores computed TRANSPOSED S_T[k,q] k-tile-outer so the
    exp'd probabilities feed the AV matmul directly (v stationary); the
    softmax denominator comes from a vector presum of the exp'd strips
    plus one all-ones matmul that column-sums AND broadcasts in one shot;
    1/denom via DVE newton-raphson (exact reciprocal is 4us/tile and
    scalar ln/exp forces 1.3us activation-table reloads)
  * o-projection: row-parallel -> per-core partials; host sums the 8

All matmuls bf16 (f32 for the tiny denominator reduction) with f32 PSUM.
Self-contained: hardcodes all shapes; no sibling imports.
"""

import math

import numpy as np
import ml_dtypes

B, S, HIDDEN, NH, HD = 2, 2048, 2048, 16, 128
N_CORES = 8
HPC = NH // N_CORES          # heads per core = 2
M = HPC * HD                 # per-core projection width = 256
T = B * S                    # 4096 tokens
P = 128                      # partitions
TCH = 512                    # token / free-dim chunk
NTCH = T // TCH              # 8
QT = S // P                  # 16 token tiles per batch
KI = HIDDEN // P             # 16 contraction tiles for projections
NQ = 4                       # query quarters per (b,h); each 4 tiles = 512 tok
BF16 = ml_dtypes.bfloat16

_nc_cache = {}


def _build_nc():
    import concourse.bacc as bacc
    import concourse.mybir as mybir
    from concourse import tile
    from contextlib import ExitStack

    bf = mybir.dt.bfloat16
    f32 = mybir.dt.float32
    AF = mybir.ActivationFunctionType

    nc = bacc.Bacc("TRN2", target_bir_lowering=False, debug=False)

    # inputs host-prearranged to partition-major so DMA descriptors are
    # 4-16KB contiguous per partition (strided loads run ~3x slower)
    hsT = nc.dram_tensor("hsT", [P, NTCH, KI, TCH], bf, kind="ExternalInput").ap()
    wqT = nc.dram_tensor("wqT", [P, KI, M], bf, kind="ExternalInput").ap()
    wkT = nc.dram_tensor("wkT", [P, KI, M], bf, kind="ExternalInput").ap()
    wvT = nc.dram_tensor("wvT", [P, KI, M], bf, kind="ExternalInput").ap()
    woT = nc.dram_tensor("woT", [P, HPC, HIDDEN], bf, kind="ExternalInput").ap()
    msk = nc.dram_tensor("mask", [P, P], f32, kind="ExternalInput").ap()
    out = nc.dram_tensor("out", [T, HIDDEN], bf, kind="ExternalOutput").ap()

    out_r = out.rearrange("(n p) o -> p n o", p=P)      # [128, 32, 2048]

    inv_sqrt_d = 1.0 / math.sqrt(HD)

    with tile.TileContext(nc) as tc, ExitStack() as ctx:
        const = ctx.enter_context(tc.tile_pool(name="const", bufs=1))
        qkv = ctx.enter_context(tc.tile_pool(name="qkv", bufs=1))
        hsp = ctx.enter_context(tc.tile_pool(name="hsp", bufs=3))
        ppl = ctx.enter_context(tc.tile_pool(name="ppl", bufs=6))
        pac = ctx.enter_context(tc.tile_pool(name="pac", bufs=2))
        rcp = ctx.enter_context(tc.tile_pool(name="rcp", bufs=2))
        orp = ctx.enter_context(tc.tile_pool(name="orp", bufs=3))
        pp = ctx.enter_context(tc.tile_pool(name="pp", bufs=2, space="PSUM"))
        stp = ctx.enter_context(tc.tile_pool(name="stp", bufs=3, space="PSUM"))
        avp = ctx.enter_context(tc.tile_pool(name="avp", bufs=2, space="PSUM"))
        bcp = ctx.enter_context(tc.tile_pool(name="bcp", bufs=1, space="PSUM"))

        # --- constants / weights resident in SBUF ---
        # DMA order tuned for startup: wq first (chunk 0's q matmuls need
        # only wq + first hs half); wk/wv after chunk 0's hs DMA; wo (only
        # needed by phase 3) deferred until after chunk 0 is emitted.
        wq_sb = const.tile([P, KI, M], bf)
        wk_sb = const.tile([P, KI, M], bf)
        wv_sb = const.tile([P, KI, M], bf)
        wo_sb = const.tile([P, HPC, HIDDEN], bf)
        msk_sb = const.tile([P, P], f32)
        ones_sb = const.tile([P, P], bf)
        for i4 in range(0, KI, 4):
            nc.sync.dma_start(wq_sb[:, i4:i4 + 4], wqT[:, i4:i4 + 4])
        nc.vector.memset(ones_sb[:], 1.0)

        # --- persistent activations ---
        qT_b = [qkv.tile([P, HPC, S], bf, tag=f"qT{b}", name=f"qT{b}") for b in range(B)]
        kT_b = [qkv.tile([P, HPC, S], bf, tag=f"kT{b}", name=f"kT{b}") for b in range(B)]
        vT_b = [qkv.tile([P, HPC, S], bf, tag=f"vT{b}", name=f"vT{b}") for b in range(B)]
        cxT_b = [qkv.tile([P, HPC, S], bf, tag=f"cxT{b}", name=f"cxT{b}") for b in range(B)]
        # v natural layout: [tok%128, head, tile, d]
        v_b = [qkv.tile([P, HPC, QT, P], bf, tag=f"v{b}", name=f"v{b}") for b in range(B)]

        # ---- Phase 1 emitter: one 512-token chunk; yields per psum-group ----
        def p1_chunk(j):
            hs_t = hsp.tile([P, KI, TCH], bf, tag="hs", name="hs_t")
            # chunk 0: finely split on the second hwdge queue so the first
            # matmul starts after a quarter of wq + a quarter of hs
            if j == 0:
                for i4 in range(0, KI, 4):
                    nc.scalar.dma_start(hs_t[:, i4:i4 + 4], hsT[:, j, i4:i4 + 4])
            else:
                nc.sync.dma_start(hs_t[:, :KI // 2], hsT[:, j, :KI // 2])
                nc.sync.dma_start(hs_t[:, KI // 2:], hsT[:, j, KI // 2:])
            b = j // 4
            joff = (j % 4) * TCH
            # q projection (scale 1/sqrt(d) folded into the copy)
            for h in range(HPC):
                ps = pp.tile([P, TCH], f32, tag="pp", name="ps_q")
                for i in range(KI):
                    nc.tensor.matmul(
                        ps[:], wq_sb[:, i, h * P:(h + 1) * P], hs_t[:, i, :],
                        start=(i == 0), stop=(i == KI - 1),
                    )
                if h == 0:
                    nc.scalar.mul(qT_b[b][:, h, joff:joff + TCH], ps[:], inv_sqrt_d)
                else:
                    nc.vector.tensor_scalar_mul(
                        qT_b[b][:, h, joff:joff + TCH], ps[:], inv_sqrt_d)
                yield
            # k projection
            for h in range(HPC):
                ps = pp.tile([P, TCH], f32, tag="pp", name="ps_k")
                for i in range(KI):
                    nc.tensor.matmul(
                        ps[:], wk_sb[:, i, h * P:(h + 1) * P], hs_t[:, i, :],
                        start=(i == 0), stop=(i == KI - 1),
                    )
                if h == 0:
                    nc.scalar.copy(kT_b[b][:, h, joff:joff + TCH], ps[:])
                else:
                    nc.vector.tensor_copy(kT_b[b][:, h, joff:joff + TCH], ps[:])
                yield
            # v projection in [d, tok]; then XBAR transpose to [tok, d]
            for h in range(HPC):
                ps = pp.tile([P, TCH], f32, tag="pp", name="ps_v")
                for i in range(KI):
                    nc.tensor.matmul(
                        ps[:], wv_sb[:, i, h * P:(h + 1) * P], hs_t[:, i, :],
                        start=(i == 0), stop=(i == KI - 1),
                    )
                if h == 0:
                    nc.scalar.copy(vT_b[b][:, h, joff:joff + TCH], ps[:])
                else:
                    nc.vector.tensor_copy(vT_b[b][:, h, joff:joff + TCH], ps[:])
                nc.sync.dma_start_transpose(
                    v_b[b][:, h, (j % 4) * 4:(j % 4) * 4 + 4, :],
                    vT_b[b][:, h, joff:joff + TCH],
                )
                yield

        # ---- Phase 2 emitter: attention for (b, h); yields per k-tile.
        # fine_finalize: normalize per 128-token group as its denominator
        # completes (used on the last quarter so phase 3 unlocks early) ----
        def attn_quarter(b, h, Q, fine_finalize=False):
            qT, kT, cxT, v = qT_b[b], kT_b[b], cxT_b[b], v_b[b]
            if True:
                q0 = Q * 4 * P                       # quarter col offset (tokens)
                p_acc = pac.tile([P, TCH], f32, tag="pacc", name="p_acc")
                p_accb = pac.tile([P, TCH], bf, tag="paccb", name="p_accb")
                av = avp.tile([P, TCH], f32, tag="av", name="av")
                p_prev = None                        # deferred strip for pairing
                t_first = None                       # first pair-sum, fused below
                acc_live = False                     # p_acc initialized yet?
                nkb = 4 * Q + 4
                pend = []                            # skewed AV emission
                for kb in range(nkb):
                    c0 = max(kb - 4 * Q, 0) * P      # first live col in quarter
                    st = stp.tile([P, TCH], f32, tag="st", name="st")
                    nc.tensor.matmul(
                        st[:, c0:], kT[:, h, kb * P:(kb + 1) * P],
                        qT[:, h, q0 + c0:q0 + 4 * P],
                        start=True, stop=True,
                    )
                    if kb >= 4 * Q:                  # causal diag tile mask
                        nc.vector.tensor_add(st[:, c0:c0 + P], st[:, c0:c0 + P], msk_sb[:])
                    p = ppl.tile([P, TCH], bf, tag="p", name="p_t")
                    nc.scalar.activation(p[:, c0:], st[:, c0:], AF.Exp)
                    # presum: full-width strips are paired with bf16 adds
                    # (2x DVE rate); pair-sums and diag strips accumulate f32
                    if kb < 4 * Q:
                        if p_prev is None:
                            p_prev = p
                        else:
                            ts = ppl.tile([P, TCH], bf, tag="ts", name="ts", bufs=2)
                            nc.vector.tensor_add(ts[:], p_prev[:], p[:])
                            p_prev = None
                            if t_first is None:
                                t_first = ts
                            elif not acc_live:
                                nc.vector.tensor_add(p_acc[:], t_first[:], ts[:])
                                acc_live = True
                            else:
                                nc.vector.tensor_add(p_acc[:], p_acc[:], ts[:])
                    elif t_first is not None and not acc_live:
                        # Q1: single pair-sum; fold it in with this diag strip
                        nc.vector.tensor_add(p_acc[:], t_first[:], p[:])
                        acc_live = True
                    elif not acc_live:
                        # Q0: no pairs; first two diag strips fuse
                        if p_prev is None:
                            p_prev = p
                        else:
                            nc.vector.tensor_add(p_acc[:, c0:], p_prev[:, c0:], p[:, c0:])
                            if c0:
                                nc.vector.tensor_copy(p_acc[:, :c0], p_prev[:, :c0])
                            p_prev = None
                            acc_live = True
                    else:
                        nc.vector.tensor_add(p_acc[:, c0:], p_acc[:, c0:], p[:, c0:])
                    if len(pend) >= 1:
                        yield from pend.pop(0)
                    def av_mm(kb=kb, c0=c0, p=p):
                        if fine_finalize and kb >= 4 * Q:
                            # group kb's av column range is final here
                            nc.tensor.matmul(
                                av[:, c0:c0 + P], v[:, h, kb, :], p[:, c0:c0 + P],
                                start=(kb == 0), stop=True,
                            )
                            if c0 + P < TCH:
                                nc.tensor.matmul(
                                    av[:, c0 + P:], v[:, h, kb, :], p[:, c0 + P:],
                                    start=(kb == 0), stop=False,
                                )
                            finalize_group(kb - 4 * Q)
                        else:
                            nc.tensor.matmul(
                                av[:, c0:], v[:, h, kb, :], p[:, c0:],
                                start=(kb == 0), stop=(kb == nkb - 1),
                            )
                        yield
                    pend.append(av_mm())
                    if fine_finalize and kb == 4 * Q:
                        bc = bcp.tile([P, TCH], f32, tag="bc", name="bc")
                        rc = rcp.tile([P, TCH], f32, tag="rc", name="rc")

                        def finalize_group(g):
                            c = g * P
                            nc.scalar.copy(p_accb[:, c:c + P], p_acc[:, c:c + P])
                            nc.tensor.matmul(bc[:, c:c + P], ones_sb[:],
                                             p_accb[:, c:c + P], start=True, stop=True)
                            nc.vector.reciprocal_approx_fast(rc[:, c:c + P], bc[:, c:c + P])
                            nc.vector.tensor_mul(
                                cxT[:, h, q0 + c:q0 + c + P], av[:, c:c + P], rc[:, c:c + P])
                for g in pend:
                    yield from g
                if not fine_finalize:
                    # denominator: column-sum + partition-broadcast in one
                    # matmul; 1/d via single-op newton-raphson on DVE; bf16
                    # cast on scalar so the colsum matmul streams at bf16 rate
                    nc.scalar.copy(p_accb[:], p_acc[:])
                    bc = bcp.tile([P, TCH], f32, tag="bc", name="bc")
                    nc.tensor.matmul(bc[:], ones_sb[:], p_accb[:], start=True, stop=True)
                    rc = rcp.tile([P, TCH], f32, tag="rc", name="rc")
                    nc.vector.reciprocal_approx_fast(rc[:], bc[:])
                    nc.vector.tensor_mul(cxT[:, h, q0:q0 + 4 * P], av[:], rc[:])
                yield

        def attn_pair(b, h, fine_last=False):
            for Q in range(NQ):
                yield from attn_quarter(b, h, Q,
                                        fine_finalize=(fine_last and Q == NQ - 1))

        # ---- Phase 3 emitter: o-projection rows; yields per token tile ----
        def p3_rows(b):
            for tloc in range(QT):
                orow = orp.tile([P, HIDDEN], bf, tag="orow", name="orow")
                for oc in range(HIDDEN // TCH):
                    ps = pp.tile([P, TCH], f32, tag="pp", name="ps_o")
                    for h in range(HPC):
                        nc.tensor.matmul(
                            ps[:], cxT_b[b][:, h, tloc * P:(tloc + 1) * P],
                            wo_sb[:, h, oc * TCH:(oc + 1) * TCH],
                            start=(h == 0), stop=(h == HPC - 1),
                        )
                    if oc % 2 == 0:
                        nc.scalar.copy(orow[:, oc * TCH:(oc + 1) * TCH], ps[:])
                    else:
                        nc.vector.tensor_copy(orow[:, oc * TCH:(oc + 1) * TCH], ps[:])
                nc.sync.dma_start(out_r[:, b * QT + tloc, :], orow[:])
                yield

        def run(gen):
            for _ in gen:
                pass

        def interleave(main, filler, ratio):
            """Drive main; after every `ratio` main steps, one filler step."""
            n = 0
            for _ in main:
                n += 1
                if filler is not None and n % ratio == 0:
                    try:
                        next(filler)
                    except StopIteration:
                        filler = None
            while filler is not None:
                try:
                    next(filler)
                except StopIteration:
                    filler = None

        def interleave_gated(main, filler, gates, spread=2):
            """Drive main; `gates[n]` releases that many filler steps once
            main has taken n steps (at most one per `spread` main steps)."""
            n, budget = 0, 0
            for _ in main:
                n += 1
                budget += gates.get(n, 0)
                if filler is not None and budget > 0 and n % spread == 0:
                    budget -= 1
                    try:
                        next(filler)
                    except StopIteration:
                        filler = None
            while filler is not None:
                try:
                    next(filler)
                except StopIteration:
                    filler = None

        def chain(*gens):
            for g in gens:
                yield from g

        # schedule: P1(b0); A(b0)+P1(b1); A(b1 both heads, quarter-interleaved,
        # with h1's last quarter BEFORE h0's so phase 3 tiles unlock early)+P3
        # chunk 0 starts on wq + its first hs half; wk/wv land during its
        # q-groups, wo during chunk 1
        g0 = p1_chunk(0)
        next(g0)
        next(g0)
        nc.sync.dma_start(wk_sb[:], wkT)
        nc.sync.dma_start(wv_sb[:], wvT)
        nc.sync.dma_start(msk_sb[:], msk)
        run(g0)
        nc.sync.dma_start(wo_sb[:], woT)
        for j in range(1, 4):
            run(p1_chunk(j))
        interleave(chain(attn_pair(0, 0), attn_pair(0, 1)),
                   chain(*[p1_chunk(j) for j in range(4, 8)]), ratio=3)
        interleave(attn_pair(1, 0), p3_rows(0), ratio=3)
        # p3(b1) token tiles 4Q..4Q+3 unlock after quarter Q of the last
        # pair; the last quarter finalizes per 128-token group (yields 41-44)
        interleave_gated(attn_pair(1, 1, fine_last=True), p3_rows(1),
                         gates={5: 4, 14: 4, 27: 4, 41: 1, 42: 1, 43: 1, 44: 1},
                         spread=1)

    nc.compile()
    return nc


def get_nc():
    if "nc" not in _nc_cache:
        _nc_cache["nc"] = _build_nc()
    return _nc_cache["nc"]


def _warr(wT):
    """[HIDDEN, M] transposed weight -> partition-major [P, KI, M]."""
    return np.ascontiguousarray(
        wT.reshape(KI, P, M).transpose(1, 0, 2)).astype(BF16)


def make_in_maps(hidden_states, wq, wk, wv, wo):
    hs = np.asarray(hidden_states, dtype=np.float32).reshape(T, HIDDEN)
    # [hid, tok] -> partition-major chunks [P, NTCH, KI, TCH]
    hsT = np.ascontiguousarray(
        hs.T.reshape(KI, P, NTCH, TCH).transpose(1, 2, 0, 3)).astype(BF16)
    # S_T[k, q] layout: mask out k > q (strictly lower triangle)
    mask = np.tril(np.full((P, P), -1e9, dtype=np.float32), -1)
    wq = np.asarray(wq, dtype=np.float32)
    wk = np.asarray(wk, dtype=np.float32)
    wv = np.asarray(wv, dtype=np.float32)
    wo = np.asarray(wo, dtype=np.float32)
    in_maps = []
    for c in range(N_CORES):
        sl = slice(c * M, (c + 1) * M)
        woc = np.ascontiguousarray(
            wo[:, sl].T.reshape(HPC, P, HIDDEN).transpose(1, 0, 2)).astype(BF16)
        in_maps.append({
            "hsT": hsT,
            "wqT": _warr(wq[sl, :].T),
            "wkT": _warr(wk[sl, :].T),
            "wvT": _warr(wv[sl, :].T),
            "woT": woc,
            "mask": mask,
        })
    return in_maps


def kernel(hidden_states, wq, wk, wv, wo):
    from concourse.bass_utils import run_bass_kernel_spmd

    nc = get_nc()
    in_maps = make_in_maps(hidden_states, wq, wk, wv, wo)
    res = run_bass_kernel_spmd(nc, in_maps, core_ids=list(range(N_CORES)))
    acc = np.zeros((T, HIDDEN), dtype=np.float32)
    for r in res.results:
        acc += np.asarray(r["out"]).astype(np.float32)
    return acc.reshape(B, S, HIDDEN)


# revision 59
# speedup vs baseline: 1.1979x; 1.0006x over previous
"""Distributed LlamaAttention (B=2, S=2048, H=2048, 16 heads) on one TRN2 chip.

Sharding: tensor-parallel over heads - core c owns heads (2c, 2c+1).
  * q/k/v projections -> qT/kT/vT [d, tok] (weight-stationary, N=512)
  * v to natural [tok, d] layout via XBAR DMA transpose (no PE cost)
  * attention: scores computed TRANSPOSED S_T[k,q] k-tile-outer so the
    exp'd probabilities feed the AV matmul directly (v stationary); the
    softmax denominator comes from a vector presum of the exp'd strips
    plus one all-ones matmul that column-sums AND broadcasts in one shot;
    1/denom via DVE newton-raphson (exact reciprocal is 4us/tile and
    scalar ln/exp forces 1.3us activation-table reloads)
  * o-projection: row-parallel -> per-core partials; host sums the 8

All matmuls bf16 (f32 for the tiny denominator reduction) with f32 PSUM.
Self-contained: hardcodes all shapes; no sibling imports.
"""

import math

import numpy as np
import ml_dtypes

B, S, HIDDEN, NH, HD = 2, 2048, 2048, 16, 128
N_CORES = 8
HPC = NH // N_CORES          # heads per core = 2
M = HPC * HD                 # per-core projection width = 256
T = B * S                    # 4096 tokens
P = 128                      # partitions
TCH = 512                    # token / free-dim chunk
NTCH = T // TCH              # 8
QT = S // P                  # 16 token tiles per batch
KI = HIDDEN // P             # 16 contraction tiles for projections
NQ = 4                       # query quarters per (b,h); each 4 tiles = 512 tok
BF16 = ml_dtypes.bfloat16

_nc_cache = {}


def _build_nc():
    import concourse.bacc as bacc
    import concourse.mybir as mybir
    from concourse import tile
    from contextlib import ExitStack

    bf = mybir.dt.bfloat16
    f32 = mybir.dt.float32
    AF = mybir.ActivationFunctionType

    nc = bacc.Bacc("TRN2", target_bir_lowering=False, debug=False)

    # inputs host-prearranged to partition-major so DMA descriptors are
    # 4-16KB contiguous per partition (strided loads run ~3x slower)
    hsT = nc.dram_tensor("hsT", [P, NTCH, KI, TCH], bf, kind="ExternalInput").ap()
    wqT = nc.dram_tensor("wqT", [P, KI, M], bf, kind="ExternalInput").ap()
    wkT = nc.dram_tensor("wkT", [P, KI, M], bf, kind="ExternalInput").ap()
    wvT = nc.dram_tensor("wvT", [P, KI, M], bf, kind="ExternalInput").ap()
    woT = nc.dram_tensor("woT", [P, HPC, HIDDEN], bf, kind="ExternalInput").ap()
    msk = nc.dram_tensor("mask", [P, P], f32, kind="ExternalInput").ap()
    out = nc.dram_tensor("out", [T, HIDDEN], bf, kind="ExternalOutput").ap()

    out_r = out.rearrange("(n p) o -> p n o", p=P)      # [128, 32, 2048]

    inv_sqrt_d = 1.0 / math.sqrt(HD)

    with tile.TileContext(nc) as tc, ExitStack() as ctx:
        const = ctx.enter_context(tc.tile_pool(name="const", bufs=1))
        qkv = ctx.enter_context(tc.tile_pool(name="qkv", bufs=1))
        hsp = ctx.enter_context(tc.tile_pool(name="hsp", bufs=3))
        ppl = ctx.enter_context(tc.tile_pool(name="ppl", bufs=6))
        pac = ctx.enter_context(tc.tile_pool(name="pac", bufs=2))
        rcp = ctx.enter_context(tc.tile_pool(name="rcp", bufs=2))
        orp = ctx.enter_context(tc.tile_pool(name="orp", bufs=3))
        pp = ctx.enter_context(tc.tile_pool(name="pp", bufs=2, space="PSUM"))
        stp = ctx.enter_context(tc.tile_pool(name="stp", bufs=3, space="PSUM"))
        avp = ctx.enter_context(tc.tile_pool(name="avp", bufs=2, space="PSUM"))
        bcp = ctx.enter_context(tc.tile_pool(name="bcp", bufs=1, space="PSUM"))

        # --- constants / weights resident in SBUF ---
        # DMA order tuned for startup: wq first (chunk 0's q matmuls need
        # only wq + first hs half); wk/wv after chunk 0's hs DMA; wo (only
        # needed by phase 3) deferred until after chunk 0 is emitted.
        wq_sb = const.tile([P, KI, M], bf)
        wk_sb = const.tile([P, KI, M], bf)
        wv_sb = const.tile([P, KI, M], bf)
        wo_sb = const.tile([P, HPC, HIDDEN], bf)
        msk_sb = const.tile([P, P], f32)
        ones_sb = const.tile([P, P], bf)
        for i4 in range(0, KI, 4):
            nc.sync.dma_start(wq_sb[:, i4:i4 + 4], wqT[:, i4:i4 + 4])
        nc.vector.memset(ones_sb[:], 1.0)

        # --- persistent activations ---
        qT_b = [qkv.tile([P, HPC, S], bf, tag=f"qT{b}", name=f"qT{b}") for b in range(B)]
        kT_b = [qkv.tile([P, HPC, S], bf, tag=f"kT{b}", name=f"kT{b}") for b in range(B)]
        vT_b = [qkv.tile([P, HPC, S], bf, tag=f"vT{b}", name=f"vT{b}") for b in range(B)]
        cxT_b = [qkv.tile([P, HPC, S], bf, tag=f"cxT{b}", name=f"cxT{b}") for b in range(B)]
        # v natural layout: [tok%128, head, tile, d]
        v_b = [qkv.tile([P, HPC, QT, P], bf, tag=f"v{b}", name=f"v{b}") for b in range(B)]

        # ---- Phase 1 emitter: one 512-token chunk; yields per psum-group ----
        def p1_chunk(j):
            hs_t = hsp.tile([P, KI, TCH], bf, tag="hs", name="hs_t")
            # chunk 0: finely split on the second hwdge queue so the first
            # matmul starts after a quarter of wq + a quarter of hs
            if j == 0:
                for i4 in range(0, KI, 4):
                    nc.scalar.dma_start(hs_t[:, i4:i4 + 4], hsT[:, j, i4:i4 + 4])
            else:
                nc.sync.dma_start(hs_t[:, :KI // 2], hsT[:, j, :KI // 2])
                nc.sync.dma_start(hs_t[:, KI // 2:], hsT[:, j, KI // 2:])
            b = j // 4
            joff = (j % 4) * TCH
            # q projection (scale 1/sqrt(d) folded into the copy)
            for h in range(HPC):
                ps = pp.tile([P, TCH], f32, tag="pp", name="ps_q")
                for i in range(KI):
                    nc.tensor.matmul(
                        ps[:], wq_sb[:, i, h * P:(h + 1) * P], hs_t[:, i, :],
                        start=(i == 0), stop=(i == KI - 1),
                    )
                if h == 0:
                    nc.scalar.mul(qT_b[b][:, h, joff:joff + TCH], ps[:], inv_sqrt_d)
                else:
                    nc.vector.tensor_scalar_mul(
                        qT_b[b][:, h, joff:joff + TCH], ps[:], inv_sqrt_d)
                yield
            # k projection
            for h in range(HPC):
                ps = pp.tile([P, TCH], f32, tag="pp", name="ps_k")
                for i in range(KI):
                    nc.tensor.matmul(
                        ps[:], wk_sb[:, i, h * P:(h + 1) * P], hs_t[:, i, :],
                        start=(i == 0), stop=(i == KI - 1),
                    )
                if h == 0:
                    nc.scalar.copy(kT_b[b][:, h, joff:joff + TCH], ps[:])
                else:
                    nc.vector.tensor_copy(kT_b[b][:, h, joff:joff + TCH], ps[:])
                yield
            # v projection in [d, tok]; then XBAR transpose to [tok, d]
            for h in range(HPC):
                ps = pp.tile([P, TCH], f32, tag="pp", name="ps_v")
                for i in range(KI):
                    nc.tensor.matmul(
                        ps[:], wv_sb[:, i, h * P:(h + 1) * P], hs_t[:, i, :],
                        start=(i == 0), stop=(i == KI - 1),
                    )
                if h == 0:
                    nc.scalar.copy(vT_b[b][:, h, joff:joff + TCH], ps[:])
                else:
                    nc.vector.tensor_copy(vT_b[b][:, h, joff:joff + TCH], ps[:])
                nc.sync.dma_start_transpose(
                    v_b[b][:, h, (j % 4) * 4:(j % 4) * 4 + 4, :],
                    vT_b[b][:, h, joff:joff + TCH],
                )
                yield

        # ---- Phase 2 emitter: attention for (b, h); yields per k-tile.
        # fine_finalize: normalize per 128-token group as its denominator
        # completes (used on the last quarter so phase 3 unlocks early) ----
        def attn_quarter(b, h, Q, fine_finalize=False):
            qT, kT, cxT, v = qT_b[b], kT_b[b], cxT_b[b], v_b[b]
            if True:
                q0 = Q * 4 * P                       # quarter col offset (tokens)
                p_acc = pac.tile([P, TCH], f32, tag="pacc", name="p_acc")
                p_accb = pac.tile([P, TCH], bf, tag="paccb", name="p_accb")
                av = avp.tile([P, TCH], f32, tag="av", name="av")
                p_prev = None                        # deferred strip for pairing
                t_first = None                       # first pair-sum, fused below
                acc_live = False                     # p_acc initialized yet?
                nkb = 4 * Q + 4
                pend = []                            # skewed AV emission
                for kb in range(nkb):
                    c0 = max(kb - 4 * Q, 0) * P      # first live col in quarter
                    st = stp.tile([P, TCH], f32, tag="st", name="st")
                    nc.tensor.matmul(
                        st[:, c0:], kT[:, h, kb * P:(kb + 1) * P],
                        qT[:, h, q0 + c0:q0 + 4 * P],
                        start=True, stop=True,
                    )
                    if kb >= 4 * Q:                  # causal diag tile mask
                        nc.vector.tensor_add(st[:, c0:c0 + P], st[:, c0:c0 + P], msk_sb[:])
                    p = ppl.tile([P, TCH], bf, tag="p", name="p_t")
                    nc.scalar.activation(p[:, c0:], st[:, c0:], AF.Exp)
                    # presum: full-width strips are paired with bf16 adds
                    # (2x DVE rate); pair-sums and diag strips accumulate f32
                    if kb < 4 * Q:
                        if p_prev is None:
                            p_prev = p
                        else:
                            ts = ppl.tile([P, TCH], bf, tag="ts", name="ts", bufs=2)
                            nc.vector.tensor_add(ts[:], p_prev[:], p[:])
                            p_prev = None
                            if t_first is None:
                                t_first = ts
                            elif not acc_live:
                                nc.vector.tensor_add(p_acc[:], t_first[:], ts[:])
                                acc_live = True
                            else:
                                nc.vector.tensor_add(p_acc[:], p_acc[:], ts[:])
                    elif t_first is not None and not acc_live:
                        # Q1: single pair-sum; fold it in with this diag strip
                        nc.vector.tensor_add(p_acc[:], t_first[:], p[:])
                        acc_live = True
                    elif not acc_live:
                        # Q0: no pairs; first two diag strips fuse
                        if p_prev is None:
                            p_prev = p
                        else:
                            nc.vector.tensor_add(p_acc[:, c0:], p_prev[:, c0:], p[:, c0:])
                            if c0:
                                nc.vector.tensor_copy(p_acc[:, :c0], p_prev[:, :c0])
                            p_prev = None
                            acc_live = True
                    else:
                        nc.vector.tensor_add(p_acc[:, c0:], p_acc[:, c0:], p[:, c0:])
                    if len(pend) >= 1:
                        yield from pend.pop(0)
                    def av_mm(kb=kb, c0=c0, p=p):
                        if fine_finalize and kb >= 4 * Q:
                            # group kb's av column range is final here
                            nc.tensor.matmul(
                                av[:, c0:c0 + P], v[:, h, kb, :], p[:, c0:c0 + P],
                                start=(kb == 0), stop=True,
                            )
                            if c0 + P < TCH:
                                nc.tensor.matmul(
                                    av[:, c0 + P:], v[:, h, kb, :], p[:, c0 + P:],
                                    start=(kb == 0), stop=False,
                                )
                            finalize_group(kb - 4 * Q)
                        else:
                            nc.tensor.matmul(
                                av[:, c0:], v[:, h, kb, :], p[:, c0:],
                                start=(kb == 0), stop=(kb == nkb - 1),
                            )
                        yield
                    pend.append(av_mm())
                    if fine_finalize and kb == 4 * Q:
                        bc = bcp.tile([P, TCH], f32, tag="bc", name="bc")
                        rc = rcp.tile([P, TCH], f32, tag="rc", name="rc")

                        def finalize_group(g):
                            c = g * P
                            nc.scalar.copy(p_accb[:, c:c + P], p_acc[:, c:c + P])
                            nc.tensor.matmul(bc[:, c:c + P], ones_sb[:],
                                             p_accb[:, c:c + P], start=True, stop=True)
                            nc.vector.reciprocal_approx_fast(rc[:, c:c + P], bc[:, c:c + P])
                            nc.vector.tensor_mul(
                                cxT[:, h, q0 + c:q0 + c + P], av[:, c:c + P], rc[:, c:c + P])
                for g in pend:
                    yield from g
                if not fine_finalize:
                    # denominator: column-sum + partition-broadcast in one
                    # matmul; 1/d via single-op newton-raphson on DVE; bf16
                    # cast on scalar so the colsum matmul streams at bf16 rate
                    nc.scalar.copy(p_accb[:], p_acc[:])
                    bc = bcp.tile([P, TCH], f32, tag="bc", name="bc")
                    nc.tensor.matmul(bc[:], ones_sb[:], p_accb[:], start=True, stop=True)
                    rc = rcp.tile([P, TCH], f32, tag="rc", name="rc")
                    nc.vector.reciprocal_approx_fast(rc[:], bc[:])
                    nc.vector.tensor_mul(cxT[:, h, q0:q0 + 4 * P], av[:], rc[:])
                yield

        def attn_pair(b, h, fine_last=False):
            for Q in range(NQ):
                yield from attn_quarter(b, h, Q,
                                        fine_finalize=(fine_last and Q == NQ - 1))

        # ---- Phase 3 emitter: o-projection rows; yields per token tile ----
        def p3_rows(b):
            for tloc in range(QT):
                orow = orp.tile([P, HIDDEN], bf, tag="orow", name="orow")
                for oc in range(HIDDEN // TCH):
                    ps = pp.tile([P, TCH], f32, tag="pp", name="ps_o")
                    for h in range(HPC):
                        nc.tensor.matmul(
                            ps[:], cxT_b[b][:, h, tloc * P:(tloc + 1) * P],
                            wo_sb[:, h, oc * TCH:(oc + 1) * TCH],
                            start=(h == 0), stop=(h == HPC - 1),
                        )
                    if oc % 2 == 0:
                        nc.scalar.copy(orow[:, oc * TCH:(oc + 1) * TCH], ps[:])
                    else:
                        nc.vector.tensor_copy(orow[:, oc * TCH:(oc + 1) * TCH], ps[:])
                nc.sync.dma_start(out_r[:, b * QT + tloc, :], orow[:])
                yield

        def run(gen):
            for _ in gen:
                pass

        def interleave(main, filler, ratio):
            """Drive main; after every `ratio` main steps, one filler step."""
            n = 0
            for _ in main:
                n += 1
                if filler is not None and n % ratio == 0:
                    try:
                        next(filler)
                    except StopIteration:
                        filler = None
            while filler is not None:
                try:
                    next(filler)
                except StopIteration:
                    filler = None

        def interleave_gated(main, filler, gates, spread=2):
            """Drive main; `gates[n]` releases that many filler steps once
            main has taken n steps (at most one per `spread` main steps)."""
            n, budget = 0, 0
            for _ in main:
                n += 1
                budget += gates.get(n, 0)
                if filler is not None and budget > 0 and n % spread == 0:
                    budget -= 1
                    try:
                        next(filler)
                    except StopIteration:
                        filler = None
            while filler is not None:
                try:
                    next(filler)
                except StopIteration:
                    filler = None

        def chain(*gens):
            for g in gens:
                yield from g

        # schedule: P1(b0); A(b0)+P1(b1); A(b1 both heads, quarter-interleaved,
        # with h1's last quarter BEFORE h0's so phase 3 tiles unlock early)+P3
        # chunk 0 starts on wq + its first hs half; wk/wv land during its
        # q-groups, wo during chunk 1
        g0 = p1_chunk(0)
        next(g0)
        next(g0)
        nc.sync.dma_start(wk_sb[:], wkT)
        nc.sync.dma_start(wv_sb[:], wvT)
        nc.sync.dma_start(msk_sb[:], msk)
        run(g0)
        nc.sync.dma_start(wo_sb[:], woT)
        for j in range(1, 4):
            run(p1_chunk(j))
        interleave(chain(attn_pair(0, 0), attn_pair(0, 1)),
                   chain(*[p1_chunk(j) for j in range(4, 8)]), ratio=3)
        interleave(attn_pair(1, 0), p3_rows(0), ratio=3)
        # p3(b1) token tiles 4Q..4Q+3 unlock after quarter Q of the last pair
        interleave_gated(attn_pair(1, 1), p3_rows(1),
                         gates={5: 4, 14: 4, 27: 4, 44: 4}, spread=1)

    nc.compile()
    return nc


def get_nc():
    if "nc" not in _nc_cache:
        _nc_cache["nc"] = _build_nc()
    return _nc_cache["nc"]


def _warr(wT):
    """[HIDDEN, M] transposed weight -> partition-major [P, KI, M]."""
    return np.ascontiguousarray(
        wT.reshape(KI, P, M).transpose(1, 0, 2)).astype(BF16)


def make_in_maps(hidden_states, wq, wk, wv, wo):
    hs = np.asarray(hidden_states, dtype=np.float32).reshape(T, HIDDEN)
    # [hid, tok] -> partition-major chunks [P, NTCH, KI, TCH]
    hsT = np.ascontiguousarray(
        hs.T.reshape(KI, P, NTCH, TCH).transpose(1, 2, 0, 3)).astype(BF16)
    # S_T[k, q] layout: mask out k > q (strictly lower triangle)
    mask = np.tril(np.full((P, P), -1e9, dtype=np.float32), -1)
    wq = np.asarray(wq, dtype=np.float32)
    wk = np.asarray(wk, dtype=np.float32)
    wv = np.asarray(wv, dtype=np.float32)
    wo = np.asarray(wo, dtype=np.float32)
    in_maps = []
    for c in range(N_CORES):
        sl = slice(c * M, (c + 1) * M)
        woc = np.ascontiguousarray(
            wo[:, sl].T.reshape(HPC, P, HIDDEN).transpose(1, 0, 2)).astype(BF16)
        in_maps.append({
            "hsT": hsT,
            "wqT": _warr(wq[sl, :].T),
            "wkT": _warr(wk[sl, :].T),
            "wvT": _warr(wv[sl, :].T),
            "woT": woc,
            "mask": mask,
        })
    return in_maps


def kernel(hidden_states, wq, wk, wv, wo):
    from concourse.bass_utils import run_bass_kernel_spmd

    nc = get_nc()
    in_maps = make_in_maps(hidden_states, wq, wk, wv, wo)
    res = run_bass_kernel_spmd(nc, in_maps, core_ids=list(range(N_CORES)))
    acc = np.zeros((T, HIDDEN), dtype=np.float32)
    for r in res.results:
        acc += np.asarray(r["out"]).astype(np.float32)
    return acc.reshape(B, S, HIDDEN)
